# revision 1
# baseline (speedup 1.0000x reference)
"""CrossCoder kernel for 8 Trainium2 NeuronCores (Bass/Tile, SPMD).

Math (reference):
    f     = relu(einsum('bld,ldf->bf', x, W_enc) + b_enc)     # [B, F]
    x_hat = einsum('bf,lfd->bld', f, W_dec) + b_dec           # [B, L, D]

Sharding: dict dim F=32768 split 8 ways (FL=4096 per core, tensor parallel
over latents). Each core computes its local f shard (encode) and the
partial decode sum over its latents; ReduceScatters combine the partials,
leaving each core with a distinct slice of the (LD=2048, B) transposed
output, which the host reassembles and transposes back.

Device layout is feature-major (contraction dim on SBUF partitions); batch
runs in two halves of 512 inside ONE TileContext. Collectives are emitted
in-context: RS0 after half 0 overlaps all of half 1; half 1's partial is
split in two (ld rows 0-1023 / 1024-2047) so RS1a overlaps the tail of the
decode and only RS1b (2MB) is exposed. Weights/x are host-repacked into
contiguous [128, 512] tiles so every DMA is one 256KB contiguous block.
b_dec/8 is folded in pre-collective. All matmuls are float32r (full PE
rate, ~2e-4 rel err).
"""

import numpy as np

B = 1024
L = 2
D = 1024
F = 32768
NCORES = 8
FL = F // NCORES      # 4096 latents per core
LD = L * D            # 2048
KT = LD // 128        # 16 encode k-tiles
FT = FL // 128        # 32 f-tiles per core
NB = 512              # matmul moving free dim
NH = 2                # batch halves

_CACHE = {}


def _build_nc():
    import concourse.bass as bass  # noqa: F401
    import concourse.tile as tile
    from concourse import bacc, mybir

    f32 = mybir.dt.float32
    f32r = mybir.dt.float32r

    nc = bacc.Bacc()

    xT = nc.declare_dram_parameter("xT", [NH, KT, 128, NB], f32r, isOutput=False)
    w_enc = nc.declare_dram_parameter("w_enc", [KT, FT // 4, 128, NB], f32r, isOutput=False)
    w_dec = nc.declare_dram_parameter("w_dec", [L, 2, FT, 128, NB], f32r, isOutput=False)
    b_enc = nc.declare_dram_parameter("b_enc", [128, FT], f32, isOutput=False)
    b_dec8 = nc.declare_dram_parameter("b_dec8", [128, KT], f32, isOutput=False)
    # out_sh: [0:2] = h0 ld-tiles {2i,2i+1}; [2] = h1 ld-tile i; [3] = h1 ld-tile 8+i
    out_sh = nc.declare_dram_parameter("out_sh", [4, 128, NB], f32, isOutput=True)

    # partial buffers: one per (half, l-block) so each ReduceScatter fires as
    # soon as its 8 ld-tiles are written, spreading collective traffic
    partial0 = nc.dram_tensor("partial0", [KT, 128, NB], f32)
    parts1 = [nc.dram_tensor(f"partial1{l}", [KT // 2, 128, NB], f32) for l in range(L)]
    rs0 = nc.dram_tensor("rs0", [2, 128, NB], f32)
    rss1 = [nc.dram_tensor(f"rs1{l}", [1, 128, NB], f32) for l in range(L)]

    xT_a = xT.ap()
    w_enc_a = w_enc.ap()
    w_dec_a = w_dec.ap()
    rgroups = [list(range(NCORES))]

    with tile.TileContext(nc) as tc:
        with (
            tc.tile_pool(name="xp", bufs=1) as xp,
            tc.tile_pool(name="fp", bufs=1) as fp,
            tc.tile_pool(name="we", bufs=16) as we,
            tc.tile_pool(name="wd", bufs=16) as wd,
            tc.tile_pool(name="stg", bufs=8) as stg,
            tc.tile_pool(name="bias", bufs=1) as bias,
            tc.tile_pool(name="ps", bufs=8, space="PSUM") as ps,
        ):
            benc_t = bias.tile([128, FT], f32, name="benc")
            nc.sync.dma_start(out=benc_t, in_=b_enc.ap())
            bdec_t = bias.tile([128, KT], f32, name="bdec")
            nc.sync.dma_start(out=bdec_t, in_=b_dec8.ap())

            for h in range(NH):
                x_tiles = []
                for k in range(KT):
                    xt = xp.tile([128, NB], f32r, tag=f"x{k}", name=f"x{k}")
                    nc.sync.dma_start(out=xt, in_=xT_a[h, k])
                    x_tiles.append(xt)

                # ---- encode
                f_tiles = []
                for fg in range(FT // 4):
                    pss = [
                        ps.tile([128, NB], f32, tag="ps", name=f"pse{_j}")
                        for _j in range(4)
                    ]
                    for k in range(KT):
                        wt = we.tile([128, NB], f32r, tag="we", name="wet")
                        nc.sync.dma_start(out=wt, in_=w_enc_a[k, fg])
                        for j in range(4):
                            nc.tensor.matmul(
                                pss[j],
                                wt[:, j * 128 : (j + 1) * 128],
                                x_tiles[k],
                                start=(k == 0),
                                stop=(k == KT - 1),
                            )
                    for j in range(4):
                        ft_idx = fg * 4 + j
                        ftile = fp.tile(
                            [128, NB], f32r, tag=f"f{ft_idx}", name=f"f{ft_idx}"
                        )
                        nc.scalar.activation(
                            ftile,
                            pss[j],
                            mybir.ActivationFunctionType.Relu,
                            bias=benc_t[:, ft_idx : ft_idx + 1],
                        )
                        f_tiles.append(ftile)

                # ---- decode
                for l in range(L):
                    part_a = partial0.ap() if h == 0 else parts1[l].ap()
                    base = l * 8 if h == 0 else 0
                    for dg in range(2):
                        pss = [
                            ps.tile([128, NB], f32, tag="ps", name=f"psd{_j}")
                            for _j in range(4)
                        ]
                        for fk in range(FT):
                            wt = wd.tile([128, NB], f32r, tag="wd", name="wdt")
                            nc.sync.dma_start(out=wt, in_=w_dec_a[l, dg, fk])
                            for j in range(4):
                                nc.tensor.matmul(
                                    pss[j],
                                    wt[:, j * 128 : (j + 1) * 128],
                                    f_tiles[fk],
                                    start=(fk == 0),
                                    stop=(fk == FT - 1),
                                )
                        for j in range(4):
                            ld_t = l * 8 + dg * 4 + j
                            st = stg.tile([128, NB], f32, tag="st", name="st")
                            nc.vector.tensor_scalar_add(
                                st, pss[j], bdec_t[:, ld_t : ld_t + 1]
                            )
                            nc.sync.dma_start(
                                out=part_a[base + dg * 4 + j], in_=st
                            )
                    if h == 1:
                        # this l-block's partial is complete → ReduceScatter it
                        nc.gpsimd.collective_compute(
                            "ReduceScatter",
                            mybir.AluOpType.add,
                            ins=[parts1[l][:]],
                            outs=[rss1[l][:]],
                            replica_groups=rgroups,
                        )
                if h == 0:
                    nc.gpsimd.collective_compute(
                        "ReduceScatter",
                        mybir.AluOpType.add,
                        ins=[partial0[:]],
                        outs=[rs0[:]],
                        replica_groups=rgroups,
                    )

            out_a = out_sh.ap()
            nc.gpsimd.dma_start(out=out_a[0:2], in_=rs0[:])
            nc.gpsimd.dma_start(out=out_a[2:3], in_=rss1[0][:])
            nc.gpsimd.dma_start(out=out_a[3:4], in_=rss1[1][:])

    nc.finalize()
    return nc


def _get_nc():
    if "nc" not in _CACHE:
        _CACHE["nc"] = _build_nc()
    return _CACHE["nc"]


def kernel(x, W_enc, b_enc, W_dec, b_dec):
    from concourse.bass_utils import run_bass_kernel_spmd

    x = np.asarray(x, dtype=np.float32)
    W_enc = np.asarray(W_enc, dtype=np.float32)
    b_enc = np.asarray(b_enc, dtype=np.float32)
    W_dec = np.asarray(W_dec, dtype=np.float32)
    b_dec = np.asarray(b_dec, dtype=np.float32)

    nc = _get_nc()

    # xT blocked: [h, k, p, c] with xT row k*128+p (= x.reshape(B,LD).T), col h*512+c
    xT = np.ascontiguousarray(
        x.reshape(B, LD).T.reshape(KT, 128, NH, NB).transpose(2, 0, 1, 3)
    )
    w_enc_flat = W_enc.reshape(LD, F)
    bdec8 = np.ascontiguousarray(
        (b_dec.reshape(LD) / NCORES).astype(np.float32).reshape(KT, 128).T
    )

    in_maps = []
    for i in range(NCORES):
        fsl = slice(i * FL, (i + 1) * FL)
        we_blk = np.ascontiguousarray(
            w_enc_flat[:, fsl].reshape(KT, 128, FT // 4, NB).transpose(0, 2, 1, 3)
        )
        wd_blk = np.ascontiguousarray(
            W_dec[:, fsl, :].reshape(L, FT, 128, 2, NB).transpose(0, 3, 1, 2, 4)
        )
        in_maps.append(
            {
                "xT": xT,
                "w_enc": we_blk,
                "w_dec": wd_blk,
                "b_enc": np.ascontiguousarray(b_enc[fsl].reshape(FT, 128).T),
                "b_dec8": bdec8,
            }
        )

    res = run_bass_kernel_spmd(nc, in_maps, list(range(NCORES)))
    _CACHE["last_res"] = res

    xhatT = np.empty((LD, B), dtype=np.float32)
    for i in range(NCORES):
        arr = res.results[i]["out_sh"]  # [4, 128, NB]
        xhatT[2 * i * 128 : (2 * i + 2) * 128, 0:NB] = arr[0:2].reshape(256, NB)
        xhatT[i * 128 : (i + 1) * 128, NB : 2 * NB] = arr[2]
        xhatT[(8 + i) * 128 : (9 + i) * 128, NB : 2 * NB] = arr[3]
    return np.ascontiguousarray(xhatT.T).reshape(B, L, D).astype(np.float32)



# revision 5
# speedup vs baseline: 1.0945x; 1.0945x over previous
"""CrossCoder kernel for 8 Trainium2 NeuronCores (Bass/Tile, SPMD).

Math (reference):
    f     = relu(einsum('bld,ldf->bf', x, W_enc) + b_enc)     # [B, F]
    x_hat = einsum('bf,lfd->bld', f, W_dec) + b_dec           # [B, L, D]

Sharding: dict dim F=32768 split 8 ways (FL=4096 per core, tensor parallel
over latents). Each core computes its local f shard (encode) and the
partial decode sum over its latents. Partials are exchanged with AllToAll
(cheapest 8-core collective) and summed on VectorE at the home core, so
each core finishes with its own slice of the (LD, B) transposed output.

All matmul operands are bf16 (PSUM accumulates fp32): same PE rate as
fp32r but enables Fast Weight Load (LDWEIGHTS hidden behind matmuls) and
halves DMA bytes. A2A payload is bf16 too. End-to-end rel err ~4e-3.

Batch runs in two halves of 512 inside one TileContext; each half's A2A
overlaps the next half's compute, leaving only the last (1MB bf16) A2A +
an 8-way vector sum exposed at the tail.
"""

import numpy as np

B = 1024
L = 2
D = 1024
F = 32768
NCORES = 8
FL = F // NCORES      # 4096 latents per core
LD = L * D            # 2048
KT = LD // 128        # 16 encode k-tiles
FT = FL // 128        # 32 f-tiles per core
NB = 512              # matmul moving free dim
NH = 2                # batch halves

_CACHE = {}


def _build_nc():
    import concourse.bass as bass  # noqa: F401
    import concourse.tile as tile
    from concourse import bacc, mybir

    f32 = mybir.dt.float32
    bf16 = mybir.dt.bfloat16

    nc = bacc.Bacc()

    xT = nc.declare_dram_parameter("xT", [NH, KT, 128, NB], bf16, isOutput=False)
    w_enc = nc.declare_dram_parameter("w_enc", [KT, FT // 4, 128, NB], bf16, isOutput=False)
    w_dec = nc.declare_dram_parameter("w_dec", [L, 2, FT, 128, NB], bf16, isOutput=False)
    b_enc = nc.declare_dram_parameter("b_enc", [128, FT], f32, isOutput=False)
    b_dec8 = nc.declare_dram_parameter("b_dec8", [128, KT], f32, isOutput=False)
    # out_sh[h] = this core's ld-tiles {2i, 2i+1} for batch half h
    out_sh = nc.declare_dram_parameter("out_sh", [NH, 2, 128, NB], f32, isOutput=True)

    # A2A buffers: slot j of part = partial sums for ld-tiles {2j, 2j+1}
    parts = [nc.dram_tensor(f"part{h}", [NCORES, 2, 128, NB], bf16) for h in range(NH)]
    a2as = [
        nc.dram_tensor(f"a2a{h}", [NCORES, 2, 128, NB], bf16) for h in range(NH)
    ]

    xT_a = xT.ap()
    w_enc_a = w_enc.ap()
    w_dec_a = w_dec.ap()
    out_a = out_sh.ap()
    rgroups = [list(range(NCORES))]

    with tile.TileContext(nc) as tc:
        with (
            tc.tile_pool(name="xp", bufs=1) as xp,
            tc.tile_pool(name="fp", bufs=1) as fp,
            tc.tile_pool(name="we", bufs=16) as we,
            tc.tile_pool(name="wd", bufs=16) as wd,
            tc.tile_pool(name="stg", bufs=8) as stg,
            tc.tile_pool(name="bias", bufs=1) as bias,
            tc.tile_pool(name="fin", bufs=2) as fin,
            tc.tile_pool(name="acc", bufs=4) as accp,
            tc.tile_pool(name="ps", bufs=8, space="PSUM") as ps,
        ):
            benc_t = bias.tile([128, FT], f32, name="benc")
            nc.sync.dma_start(out=benc_t, in_=b_enc.ap())
            bdec_t = bias.tile([128, KT], f32, name="bdec")
            nc.sync.dma_start(out=bdec_t, in_=b_dec8.ap())

            def finish(h):
                # sum the 8 cores' partials for my slice; write out_sh[h]
                # gpsimd queue for all finish DMAs: the sync queue is FIFO and
                # feeds the weight pipeline — a finish DMA waiting on the A2A
                # there would stall the next half's weight loads behind it.
                for u in range(2):
                    big = fin.tile([128, NCORES * NB], bf16, tag=f"fin{u}", name=f"fin{h}{u}")
                    for s in range(NCORES):
                        nc.gpsimd.dma_start(
                            out=big[:, s * NB : (s + 1) * NB], in_=a2as[h].ap()[s, u]
                        )
                    acc = accp.tile([128, NB], f32, tag=f"acc{u}", name=f"acc{h}{u}")
                    nc.vector.tensor_add(acc, big[:, 0:NB], big[:, NB : 2 * NB])
                    for s in range(2, NCORES):
                        nc.vector.tensor_add(acc, acc, big[:, s * NB : (s + 1) * NB])
                    nc.gpsimd.dma_start(out=out_a[h, u], in_=acc)

            for h in range(NH):
                x_tiles = []
                for k in range(KT):
                    xt = xp.tile([128, NB], bf16, tag=f"x{k}", name=f"x{k}")
                    nc.sync.dma_start(out=xt, in_=xT_a[h, k])
                    x_tiles.append(xt)

                # ---- encode
                f_tiles = []
                for fg in range(FT // 4):
                    pss = [
                        ps.tile([128, NB], f32, tag="ps", name=f"pse{_j}")
                        for _j in range(4)
                    ]
                    for k in range(KT):
                        wt = we.tile([128, NB], bf16, tag="we", name="wet")
                        nc.sync.dma_start(out=wt, in_=w_enc_a[k, fg])
                        for j in range(4):
                            nc.tensor.matmul(
                                pss[j],
                                wt[:, j * 128 : (j + 1) * 128],
                                x_tiles[k],
                                start=(k == 0),
                                stop=(k == KT - 1),
                            )
                    for j in range(4):
                        ft_idx = fg * 4 + j
                        ftile = fp.tile(
                            [128, NB], bf16, tag=f"f{ft_idx}", name=f"f{ft_idx}"
                        )
                        nc.scalar.activation(
                            ftile,
                            pss[j],
                            mybir.ActivationFunctionType.Relu,
                            bias=benc_t[:, ft_idx : ft_idx + 1],
                        )
                        f_tiles.append(ftile)

                # ---- decode
                part_a = parts[h].ap()
                for l in range(L):
                    for dg in range(2):
                        pss = [
                            ps.tile([128, NB], f32, tag="ps", name=f"psd{_j}")
                            for _j in range(4)
                        ]
                        for fk in range(FT):
                            wt = wd.tile([128, NB], bf16, tag="wd", name="wdt")
                            nc.sync.dma_start(out=wt, in_=w_dec_a[l, dg, fk])
                            for j in range(4):
                                nc.tensor.matmul(
                                    pss[j],
                                    wt[:, j * 128 : (j + 1) * 128],
                                    f_tiles[fk],
                                    start=(fk == 0),
                                    stop=(fk == FT - 1),
                                )
                        for j in range(4):
                            t = l * 8 + dg * 4 + j
                            st = stg.tile([128, NB], bf16, tag="st", name="st")
                            nc.vector.tensor_scalar_add(
                                st, pss[j], bdec_t[:, t : t + 1]
                            )
                            nc.sync.dma_start(out=part_a[t // 2, t % 2], in_=st)
                nc.gpsimd.collective_compute(
                    "AllToAll",
                    mybir.AluOpType.bypass,
                    ins=[parts[h][:]],
                    outs=[a2as[h][:]],
                    replica_groups=rgroups,
                )
                finish(h)

    nc.finalize()
    return nc


def _get_nc():
    if "nc" not in _CACHE:
        _CACHE["nc"] = _build_nc()
    return _CACHE["nc"]


def kernel(x, W_enc, b_enc, W_dec, b_dec):
    import ml_dtypes
    from concourse.bass_utils import run_bass_kernel_spmd

    bf16 = ml_dtypes.bfloat16
    x = np.asarray(x, dtype=np.float32)
    W_enc = np.asarray(W_enc, dtype=np.float32)
    b_enc = np.asarray(b_enc, dtype=np.float32)
    W_dec = np.asarray(W_dec, dtype=np.float32)
    b_dec = np.asarray(b_dec, dtype=np.float32)

    nc = _get_nc()

    # xT blocked: [h, k, p, c] with xT row k*128+p (= x.reshape(B,LD).T), col h*512+c
    xT = np.ascontiguousarray(
        x.reshape(B, LD).T.reshape(KT, 128, NH, NB).transpose(2, 0, 1, 3)
    ).astype(bf16)
    w_enc_flat = W_enc.reshape(LD, F)
    bdec8 = np.ascontiguousarray(
        (b_dec.reshape(LD) / NCORES).astype(np.float32).reshape(KT, 128).T
    )

    in_maps = []
    for i in range(NCORES):
        fsl = slice(i * FL, (i + 1) * FL)
        we_blk = np.ascontiguousarray(
            w_enc_flat[:, fsl].reshape(KT, 128, FT // 4, NB).transpose(0, 2, 1, 3)
        ).astype(bf16)
        wd_blk = np.ascontiguousarray(
            W_dec[:, fsl, :].reshape(L, FT, 128, 2, NB).transpose(0, 3, 1, 2, 4)
        ).astype(bf16)
        in_maps.append(
            {
                "xT": xT,
                "w_enc": we_blk,
                "w_dec": wd_blk,
                "b_enc": np.ascontiguousarray(b_enc[fsl].reshape(FT, 128).T),
                "b_dec8": bdec8,
            }
        )

    res = run_bass_kernel_spmd(nc, in_maps, list(range(NCORES)))
    _CACHE["last_res"] = res

    xhatT = np.empty((LD, B), dtype=np.float32)
    for i in range(NCORES):
        arr = res.results[i]["out_sh"]  # [NH, 2, 128, NB]
        for h in range(NH):
            xhatT[2 * i * 128 : (2 * i + 2) * 128, h * NB : (h + 1) * NB] = arr[
                h
            ].reshape(256, NB)
    return np.ascontiguousarray(xhatT.T).reshape(B, L, D).astype(np.float32)


# revision 11
# speedup vs baseline: 1.1521x; 1.0526x over previous
"""CrossCoder kernel for 8 Trainium2 NeuronCores (Bass/Tile, SPMD).

Math (reference):
    f     = relu(einsum('bld,ldf->bf', x, W_enc) + b_enc)     # [B, F]
    x_hat = einsum('bf,lfd->bld', f, W_dec) + b_dec           # [B, L, D]

Sharding: dict dim F=32768 split 8 ways (FL=4096 per core, tensor parallel
over latents). Each core computes its local f shard (encode) and the
partial decode sum over its latents. Partials are exchanged with AllToAll
(cheapest 8-core collective) and summed on VectorE at the home core, so
each core finishes with its own slice of the (LD, B) transposed output.

All matmul operands are bf16 (PSUM accumulates fp32); A2A payload bf16.
End-to-end rel err ~4e-3 (gate 2e-2).

Queue discipline: the sync (SP) HWDGE queue carries ONLY the weight
stream; x/bias/partial-stores ride the scalar (Activation) HWDGE queue;
A2A-finish DMAs ride gpsimd. Keeping the FIFO queues separated prevents
a DMA that waits on a collective from stalling the weight pipeline.

Decode groups are permuted (PERM) so all even ld-tiles finish before all
odd ld-tiles: each batch half fires two 512KB A2As, the first mid-decode.
Only the last A2A (+8-way vector sum) is exposed at the tail.
"""

import numpy as np

B = 1024
L = 2
D = 1024
F = 32768
NCORES = 8
FL = F // NCORES      # 4096 latents per core
LD = L * D            # 2048
KT = LD // 128        # 16 encode k-tiles
FT = FL // 128        # 32 f-tiles per core
NB = 512              # matmul moving free dim
NH = 2                # batch halves

# decode group g covers ld-tiles PERM[4g : 4g+4]; evens first, then odds
PERM = [0, 2, 4, 6, 8, 10, 12, 14, 1, 3, 5, 7, 9, 11, 13, 15]

_CACHE = {}


def _build_nc():
    import concourse.bass as bass  # noqa: F401
    import concourse.tile as tile
    from concourse import bacc, mybir

    f32 = mybir.dt.float32
    bf16 = mybir.dt.bfloat16

    nc = bacc.Bacc()

    xT = nc.declare_dram_parameter("xT", [NH, KT, 128, NB], bf16, isOutput=False)
    w_enc = nc.declare_dram_parameter("w_enc", [KT, FT // 4, 128, NB], bf16, isOutput=False)
    w_dec = nc.declare_dram_parameter("w_dec", [4, FT, 128, NB], bf16, isOutput=False)
    b_enc = nc.declare_dram_parameter("b_enc", [128, FT], f32, isOutput=False)
    b_dec8 = nc.declare_dram_parameter("b_dec8", [128, KT], f32, isOutput=False)
    # out_sh[h, c] = this core's ld-tile 2i+c for batch half h
    out_sh = nc.declare_dram_parameter("out_sh", [NH, 2, 128, NB], f32, isOutput=True)

    # parts[h][c][j] = partial sums for ld-tile 2j+c (c = parity)
    parts = [
        [nc.dram_tensor(f"part{h}{c}", [NCORES, 128, NB], bf16) for c in range(2)]
        for h in range(NH)
    ]
    a2as = [
        [nc.dram_tensor(f"a2a{h}{c}", [NCORES, 128, NB], bf16) for c in range(2)]
        for h in range(NH)
    ]

    xT_a = xT.ap()
    w_enc_a = w_enc.ap()
    w_dec_a = w_dec.ap()
    out_a = out_sh.ap()
    rgroups = [list(range(NCORES))]

    with tile.TileContext(nc) as tc:
        with (
            tc.tile_pool(name="xp", bufs=1) as xp,
            tc.tile_pool(name="fp", bufs=1) as fp,
            tc.tile_pool(name="we", bufs=16) as we,
            tc.tile_pool(name="wd", bufs=16) as wd,
            tc.tile_pool(name="stg", bufs=8) as stg,
            tc.tile_pool(name="bias", bufs=1) as bias,
            tc.tile_pool(name="fin", bufs=2) as fin,
            tc.tile_pool(name="acc", bufs=4) as accp,
            tc.tile_pool(name="ps", bufs=8, space="PSUM") as ps,
        ):
            benc_t = bias.tile([128, FT], f32, name="benc")
            nc.scalar.dma_start(out=benc_t, in_=b_enc.ap())
            bdec_t = bias.tile([128, KT], f32, name="bdec")
            nc.scalar.dma_start(out=bdec_t, in_=b_dec8.ap())

            def finish(h, c, wide=False):
                # sum the 8 cores' partials for my ld-tile 2i+c of half h
                engs = [nc.sync, nc.scalar, nc.gpsimd] if wide else [nc.gpsimd]
                big = fin.tile(
                    [128, NCORES * NB], bf16, tag=f"fin{c}", name=f"fin{h}{c}"
                )
                for s in range(NCORES):
                    engs[s % len(engs)].dma_start(
                        out=big[:, s * NB : (s + 1) * NB], in_=a2as[h][c].ap()[s]
                    )
                acc = accp.tile([128, NB], f32, tag=f"acc{c}", name=f"acc{h}{c}")
                nc.vector.tensor_add(acc, big[:, 0:NB], big[:, NB : 2 * NB])
                for s in range(2, NCORES):
                    nc.vector.tensor_add(acc, acc, big[:, s * NB : (s + 1) * NB])
                nc.gpsimd.dma_start(out=out_a[h, c], in_=acc)

            for h in range(NH):
                x_tiles = []
                for k in range(KT):
                    xt = xp.tile([128, NB], bf16, tag=f"x{k}", name=f"x{k}")
                    nc.scalar.dma_start(out=xt, in_=xT_a[h, k])
                    x_tiles.append(xt)

                # ---- encode
                f_tiles = []
                for fg in range(FT // 4):
                    pss = [
                        ps.tile([128, NB], f32, tag="ps", name=f"pse{_j}")
                        for _j in range(4)
                    ]
                    for k in range(KT):
                        wt = we.tile([128, NB], bf16, tag="we", name="wet")
                        nc.sync.dma_start(out=wt, in_=w_enc_a[k, fg])
                        for j in range(4):
                            nc.tensor.matmul(
                                pss[j],
                                wt[:, j * 128 : (j + 1) * 128],
                                x_tiles[k],
                                start=(k == 0),
                                stop=(k == KT - 1),
                            )
                    for j in range(4):
                        ft_idx = fg * 4 + j
                        ftile = fp.tile(
                            [128, NB], bf16, tag=f"f{ft_idx}", name=f"f{ft_idx}"
                        )
                        nc.scalar.activation(
                            ftile,
                            pss[j],
                            mybir.ActivationFunctionType.Relu,
                            bias=benc_t[:, ft_idx : ft_idx + 1],
                        )
                        f_tiles.append(ftile)

                # ---- decode (groups permuted: even ld-tiles first)
                for g in range(4):
                    pss = [
                        ps.tile([128, NB], f32, tag="ps", name=f"psd{_j}")
                        for _j in range(4)
                    ]
                    for fk in range(FT):
                        wt = wd.tile([128, NB], bf16, tag="wd", name="wdt")
                        nc.sync.dma_start(out=wt, in_=w_dec_a[g, fk])
                        for j in range(4):
                            nc.tensor.matmul(
                                pss[j],
                                wt[:, j * 128 : (j + 1) * 128],
                                f_tiles[fk],
                                start=(fk == 0),
                                stop=(fk == FT - 1),
                            )
                    for j in range(4):
                        t = PERM[4 * g + j]
                        st = stg.tile([128, NB], bf16, tag="st", name="st")
                        nc.vector.tensor_scalar_add(st, pss[j], bdec_t[:, t : t + 1])
                        nc.scalar.dma_start(out=parts[h][t % 2].ap()[t // 2], in_=st)
                    if g % 2 == 1:
                        c = g // 2
                        nc.gpsimd.collective_compute(
                            "AllToAll",
                            mybir.AluOpType.bypass,
                            ins=[parts[h][c][:]],
                            outs=[a2as[h][c][:]],
                            replica_groups=rgroups,
                        )
                        finish(h, c, wide=(h == NH - 1 and c == 1))

    nc.finalize()
    return nc


def _get_nc():
    if "nc" not in _CACHE:
        _CACHE["nc"] = _build_nc()
    return _CACHE["nc"]


def kernel(x, W_enc, b_enc, W_dec, b_dec):
    import ml_dtypes
    from concourse.bass_utils import run_bass_kernel_spmd

    bf16 = ml_dtypes.bfloat16
    x = np.asarray(x, dtype=np.float32)
    W_enc = np.asarray(W_enc, dtype=np.float32)
    b_enc = np.asarray(b_enc, dtype=np.float32)
    W_dec = np.asarray(W_dec, dtype=np.float32)
    b_dec = np.asarray(b_dec, dtype=np.float32)

    nc = _get_nc()

    # xT blocked: [h, k, p, c] with xT row k*128+p (= x.reshape(B,LD).T), col h*512+c
    xT = np.ascontiguousarray(
        x.reshape(B, LD).T.reshape(KT, 128, NH, NB).transpose(2, 0, 1, 3)
    ).astype(bf16)
    w_enc_flat = W_enc.reshape(LD, F)
    bdec8 = np.ascontiguousarray(
        (b_dec.reshape(LD) / NCORES).astype(np.float32).reshape(KT, 128).T
    )

    in_maps = []
    for i in range(NCORES):
        fsl = slice(i * FL, (i + 1) * FL)
        we_blk = np.ascontiguousarray(
            w_enc_flat[:, fsl].reshape(KT, 128, FT // 4, NB).transpose(0, 2, 1, 3)
        ).astype(bf16)
        # wd_blk[g, fk, p, j*128+q] = W_dec[t//8, i*FL+fk*128+p, (t%8)*128+q],
        # with t = PERM[4g+j]
        wdl = W_dec[:, fsl, :]  # [L, FL, D]
        wd_blk = np.empty((4, FT, 128, NB), dtype=bf16)
        for g in range(4):
            for j in range(4):
                t = PERM[4 * g + j]
                blk = wdl[t // 8, :, (t % 8) * 128 : (t % 8 + 1) * 128]  # [FL, 128]
                wd_blk[g, :, :, j * 128 : (j + 1) * 128] = blk.reshape(
                    FT, 128, 128
                ).astype(bf16)
        in_maps.append(
            {
                "xT": xT,
                "w_enc": we_blk,
                "w_dec": wd_blk,
                "b_enc": np.ascontiguousarray(b_enc[fsl].reshape(FT, 128).T),
                "b_dec8": bdec8,
            }
        )

    res = run_bass_kernel_spmd(nc, in_maps, list(range(NCORES)))
    _CACHE["last_res"] = res

    xhatT = np.empty((LD, B), dtype=np.float32)
    for i in range(NCORES):
        arr = res.results[i]["out_sh"]  # [NH, 2(parity), 128, NB]
        for h in range(NH):
            xhatT[2 * i * 128 : (2 * i + 1) * 128, h * NB : (h + 1) * NB] = arr[h, 0]
            xhatT[(2 * i + 1) * 128 : (2 * i + 2) * 128, h * NB : (h + 1) * NB] = arr[
                h, 1
            ]
    return np.ascontiguousarray(xhatT.T).reshape(B, L, D).astype(np.float32)


# revision 14
# speedup vs baseline: 1.1619x; 1.0086x over previous
"""CrossCoder kernel for 8 Trainium2 NeuronCores (Bass/Tile, SPMD).

Math (reference):
    f     = relu(einsum('bld,ldf->bf', x, W_enc) + b_enc)     # [B, F]
    x_hat = einsum('bf,lfd->bld', f, W_dec) + b_dec           # [B, L, D]

Sharding: dict dim F=32768 split 8 ways (FL=4096 per core, tensor parallel
over latents). Each core computes its local f shard (encode) and the
partial decode sum over its latents. Partials are exchanged with AllToAll
(cheapest 8-core collective) and summed on VectorE at the home core, so
each core finishes with its own slice of the (LD, B) transposed output.

All matmul operands are bf16 (PSUM accumulates fp32); A2A payload bf16.
End-to-end rel err ~4e-3 (gate 2e-2).

Queue discipline: the sync (SP) HWDGE queue carries ONLY the weight
stream; x/bias/partial-stores ride the scalar (Activation) HWDGE queue;
A2A-finish DMAs ride gpsimd. Keeping the FIFO queues separated prevents
a DMA that waits on a collective from stalling the weight pipeline.

Decode groups are permuted (PERM) so all even ld-tiles finish before all
odd ld-tiles: each batch half fires two 512KB A2As, the first mid-decode.
Only the last A2A (+8-way vector sum) is exposed at the tail.
"""

import numpy as np

B = 1024
L = 2
D = 1024
F = 32768
NCORES = 8
FL = F // NCORES      # 4096 latents per core
LD = L * D            # 2048
KT = LD // 128        # 16 encode k-tiles
FT = FL // 128        # 32 f-tiles per core
NB = 512              # matmul moving free dim
NH = 2                # batch halves

# decode group g covers ld-tiles PERM[4g : 4g+4]; evens first, then odds
PERM = [0, 2, 4, 6, 8, 10, 12, 14, 1, 3, 5, 7, 9, 11, 13, 15]

_CACHE = {}


def _build_nc():
    import concourse.bass as bass  # noqa: F401
    import concourse.tile as tile
    from concourse import bacc, mybir

    f32 = mybir.dt.float32
    bf16 = mybir.dt.bfloat16

    nc = bacc.Bacc()

    xT = nc.declare_dram_parameter("xT", [NH, KT, 128, NB], bf16, isOutput=False)
    w_enc = nc.declare_dram_parameter("w_enc", [KT, FT // 4, 128, NB], bf16, isOutput=False)
    w_dec = nc.declare_dram_parameter("w_dec", [4, FT, 128, NB], bf16, isOutput=False)
    b_enc = nc.declare_dram_parameter("b_enc", [128, FT], f32, isOutput=False)
    b_dec8 = nc.declare_dram_parameter("b_dec8", [128, KT], f32, isOutput=False)
    # out_sh[h, c] = this core's ld-tile 2i+c for batch half h
    out_sh = nc.declare_dram_parameter("out_sh", [NH, 2, 128, NB], f32, isOutput=True)

    # parts[h][c][j] = partial sums for ld-tile 2j+c (c = parity)
    parts = [
        [nc.dram_tensor(f"part{h}{c}", [NCORES, 128, NB], bf16) for c in range(2)]
        for h in range(NH)
    ]
    a2as = [
        [nc.dram_tensor(f"a2a{h}{c}", [NCORES, 128, NB], bf16) for c in range(2)]
        for h in range(NH)
    ]

    xT_a = xT.ap()
    w_enc_a = w_enc.ap()
    w_dec_a = w_dec.ap()
    out_a = out_sh.ap()
    rgroups = [list(range(NCORES))]

    with tile.TileContext(nc) as tc:
        with (
            tc.tile_pool(name="xp", bufs=1) as xp,
            tc.tile_pool(name="fp", bufs=1) as fp,
            tc.tile_pool(name="we", bufs=16) as we,
            tc.tile_pool(name="wd", bufs=16) as wd,
            tc.tile_pool(name="stg", bufs=8) as stg,
            tc.tile_pool(name="bias", bufs=1) as bias,
            tc.tile_pool(name="fin", bufs=2) as fin,
            tc.tile_pool(name="acc", bufs=4) as accp,
            tc.tile_pool(name="ps", bufs=8, space="PSUM") as ps,
        ):
            benc_t = bias.tile([128, FT], f32, name="benc")
            nc.scalar.dma_start(out=benc_t, in_=b_enc.ap())
            bdec_t = bias.tile([128, KT], f32, name="bdec")
            nc.scalar.dma_start(out=bdec_t, in_=b_dec8.ap())

            def finish(h, c, wide=False):
                # sum the 8 cores' partials for my ld-tile 2i+c of half h;
                # two add-chains (VectorE + Pool) halve the serial tail
                engs = [nc.sync, nc.scalar, nc.gpsimd] if wide else [nc.gpsimd]
                big = fin.tile(
                    [128, NCORES * NB], bf16, tag=f"fin{c}", name=f"fin{h}{c}"
                )
                for s in range(NCORES):
                    engs[s % len(engs)].dma_start(
                        out=big[:, s * NB : (s + 1) * NB], in_=a2as[h][c].ap()[s]
                    )
                acc = accp.tile([128, NB], f32, tag=f"acc{c}", name=f"acc{h}{c}")
                ac2 = accp.tile([128, NB], f32, tag=f"ac2{c}", name=f"ac2{h}{c}")
                nc.vector.tensor_add(acc, big[:, 0:NB], big[:, NB : 2 * NB])
                nc.gpsimd.tensor_add(ac2, big[:, 4 * NB : 5 * NB], big[:, 5 * NB : 6 * NB])
                for s in (2, 3):
                    nc.vector.tensor_add(acc, acc, big[:, s * NB : (s + 1) * NB])
                for s in (6, 7):
                    nc.gpsimd.tensor_add(ac2, ac2, big[:, s * NB : (s + 1) * NB])
                nc.vector.tensor_add(acc, acc, ac2)
                nc.gpsimd.dma_start(out=out_a[h, c], in_=acc)

            for h in range(NH):
                x_tiles = []
                for k in range(KT):
                    xt = xp.tile([128, NB], bf16, tag=f"x{k}", name=f"x{k}")
                    nc.scalar.dma_start(out=xt, in_=xT_a[h, k])
                    x_tiles.append(xt)

                # ---- encode
                f_tiles = []
                for fg in range(FT // 4):
                    pss = [
                        ps.tile([128, NB], f32, tag="ps", name=f"pse{_j}")
                        for _j in range(4)
                    ]
                    for k in range(KT):
                        wt = we.tile([128, NB], bf16, tag="we", name="wet")
                        nc.sync.dma_start(out=wt, in_=w_enc_a[k, fg])
                        for j in range(4):
                            nc.tensor.matmul(
                                pss[j],
                                wt[:, j * 128 : (j + 1) * 128],
                                x_tiles[k],
                                start=(k == 0),
                                stop=(k == KT - 1),
                            )
                    for j in range(4):
                        ft_idx = fg * 4 + j
                        ftile = fp.tile(
                            [128, NB], bf16, tag=f"f{ft_idx}", name=f"f{ft_idx}"
                        )
                        nc.scalar.activation(
                            ftile,
                            pss[j],
                            mybir.ActivationFunctionType.Relu,
                            bias=benc_t[:, ft_idx : ft_idx + 1],
                        )
                        f_tiles.append(ftile)

                # ---- decode (groups permuted: even ld-tiles first)
                for g in range(4):
                    pss = [
                        ps.tile([128, NB], f32, tag="ps", name=f"psd{_j}")
                        for _j in range(4)
                    ]
                    for fk in range(FT):
                        wt = wd.tile([128, NB], bf16, tag="wd", name="wdt")
                        nc.sync.dma_start(out=wt, in_=w_dec_a[g, fk])
                        for j in range(4):
                            nc.tensor.matmul(
                                pss[j],
                                wt[:, j * 128 : (j + 1) * 128],
                                f_tiles[fk],
                                start=(fk == 0),
                                stop=(fk == FT - 1),
                            )
                    for j in range(4):
                        t = PERM[4 * g + j]
                        st = stg.tile([128, NB], bf16, tag="st", name="st")
                        nc.vector.tensor_scalar_add(st, pss[j], bdec_t[:, t : t + 1])
                        nc.scalar.dma_start(out=parts[h][t % 2].ap()[t // 2], in_=st)
                    if g % 2 == 1:
                        c = g // 2
                        nc.gpsimd.collective_compute(
                            "AllToAll",
                            mybir.AluOpType.bypass,
                            ins=[parts[h][c][:]],
                            outs=[a2as[h][c][:]],
                            replica_groups=rgroups,
                        )
                        finish(h, c, wide=(h == NH - 1 and c == 1))

    nc.finalize()
    return nc


def _get_nc():
    if "nc" not in _CACHE:
        _CACHE["nc"] = _build_nc()
    return _CACHE["nc"]


def kernel(x, W_enc, b_enc, W_dec, b_dec):
    import ml_dtypes
    from concourse.bass_utils import run_bass_kernel_spmd

    bf16 = ml_dtypes.bfloat16
    x = np.asarray(x, dtype=np.float32)
    W_enc = np.asarray(W_enc, dtype=np.float32)
    b_enc = np.asarray(b_enc, dtype=np.float32)
    W_dec = np.asarray(W_dec, dtype=np.float32)
    b_dec = np.asarray(b_dec, dtype=np.float32)

    nc = _get_nc()

    # xT blocked: [h, k, p, c] with xT row k*128+p (= x.reshape(B,LD).T), col h*512+c
    xT = np.ascontiguousarray(
        x.reshape(B, LD).T.reshape(KT, 128, NH, NB).transpose(2, 0, 1, 3)
    ).astype(bf16)
    w_enc_flat = W_enc.reshape(LD, F)
    bdec8 = np.ascontiguousarray(
        (b_dec.reshape(LD) / NCORES).astype(np.float32).reshape(KT, 128).T
    )

    in_maps = []
    for i in range(NCORES):
        fsl = slice(i * FL, (i + 1) * FL)
        we_blk = np.ascontiguousarray(
            w_enc_flat[:, fsl].reshape(KT, 128, FT // 4, NB).transpose(0, 2, 1, 3)
        ).astype(bf16)
        # wd_blk[g, fk, p, j*128+q] = W_dec[t//8, i*FL+fk*128+p, (t%8)*128+q],
        # with t = PERM[4g+j]
        wdl = W_dec[:, fsl, :]  # [L, FL, D]
        wd_blk = np.empty((4, FT, 128, NB), dtype=bf16)
        for g in range(4):
            for j in range(4):
                t = PERM[4 * g + j]
                blk = wdl[t // 8, :, (t % 8) * 128 : (t % 8 + 1) * 128]  # [FL, 128]
                wd_blk[g, :, :, j * 128 : (j + 1) * 128] = blk.reshape(
                    FT, 128, 128
                ).astype(bf16)
        in_maps.append(
            {
                "xT": xT,
                "w_enc": we_blk,
                "w_dec": wd_blk,
                "b_enc": np.ascontiguousarray(b_enc[fsl].reshape(FT, 128).T),
                "b_dec8": bdec8,
            }
        )

    res = run_bass_kernel_spmd(nc, in_maps, list(range(NCORES)))
    _CACHE["last_res"] = res

    xhatT = np.empty((LD, B), dtype=np.float32)
    for i in range(NCORES):
        arr = res.results[i]["out_sh"]  # [NH, 2(parity), 128, NB]
        for h in range(NH):
            xhatT[2 * i * 128 : (2 * i + 1) * 128, h * NB : (h + 1) * NB] = arr[h, 0]
            xhatT[(2 * i + 1) * 128 : (2 * i + 2) * 128, h * NB : (h + 1) * NB] = arr[
                h, 1
            ]
    return np.ascontiguousarray(xhatT.T).reshape(B, L, D).astype(np.float32)
